# revision 55
# baseline (speedup 1.0000x reference)
"""Multi-head self-attention (B=2, L=2048, D=768, H=12) on 8 TRN2 cores.

Sharding: data-parallel over batch (2 groups of 4 cores), tensor-parallel
over heads within each group (3 heads/core).  Each core computes the KQ
projection for its heads (packed into 5 half-dense 128-row chunks), V' in
[key, dim] orientation, full softmax attention, and a row-parallel partial
of the output projection.  The host sums the 4 partials per batch
(the row-parallel all-reduce) and adds the output bias.

qkv biases cost no device matmuls: the k-bias cancels in softmax, the
q-bias rides as extra V'-proj columns fed to exp as a per-partition
activation bias, and the v-bias folds into bout on the host (softmax rows
sum to 1).  All matmuls run in bf16 with fp32 PSUM accumulation; softmax
exp runs in fp32 on the scalar engine; the output partials are written in
bf16.  The softmax normalization hides its reciprocal-broadcast chain
under the next head's score loop.
Measured end-to-end L2 relative error vs the fp32 reference: ~5.7e-3.
"""

import sys

sys.path.insert(0, "/opt/trn_rl_repo")

import numpy as np
import ml_dtypes

import concourse.bass as bass
import concourse.mybir as mybir
import concourse.tile as tile
from concourse.bass_utils import run_bass_kernel_spmd
from concourse.masks import make_identity

B, L, D = 2, 2048, 768
H, HD = 12, 64
NCORES = 8
GROUPS = 4          # cores per batch
NH = H // GROUPS    # heads per core
M = NH * HD         # 192: packed width of one section (K/Q/V)
M2 = M + NH         # V'-proj width incl. the 3 q-bias columns
# Packed KQ layout (per core), 5 chunks of 128 rows:
#   chunk 0: [K0 | 0]  chunk 1: [0 | K1]  chunk 2: [K2 | 0]
#   chunk 3: [Q0 | Q1] (dense)            chunk 4: [Q2 | 0]
# Scores for head j contract over K=128 partitions (the PE runs K=64
# matmuls at half clock permanently): the zero half sits on whichever
# side (K or Q) makes the pairing contamination-free.
PACK = 5 * 128
DK = D // 128       # 6 contraction chunks
MCH = 5             # row-chunks of the packed KQ output
NQ = L // 128       # 16 query chunks
NK = L // 128       # 16 key chunks
SCALE = HD ** -0.5
BF = ml_dtypes.bfloat16
# proj emit units: (chunk, w_col_lo, n_rows, out_base_partition)
PROJ_UNITS = {
    0: (0, 0, 64, 0),      # K0
    1: (1, 192, 64, 64),   # K1 -> upper partitions
    2: (2, 256, 64, 0),    # K2
    3: (3, 384, 128, 0),   # Q0|Q1 dense
    4: (4, 512, 64, 0),    # Q2
}
KCH = {0: 0, 1: 1, 2: 2}   # scores lhsT chunk per head
QCH = {0: 3, 1: 3, 2: 4}   # scores moving chunk per head

_PROGRAM = None

# Opcodes whose walrus codegen accepts multiple sync waits (queue-level ops).
_MULTIWAIT_OK = {"EventSemaphore", "Call", "UnconditionalBranch",
                 "ConditionalBranch", "RegisterMove"}


def _split_multi_waits(nc):
    """This walrus build encodes at most ONE semaphore wait per TPB
    instruction (setupSyncWait: "Too many sync wait commands").  Tile's
    add_semaphores freely emits several.  Hoist all but one wait onto
    same-engine NoOps placed immediately before the instruction — engine
    streams execute in block order, so the stall semantics are identical.
    """
    import concourse.mybir as mybir  # local alias

    for bb in nc.main_func.blocks:
        insts = bb.instructions
        new = []
        changed = False
        for ins in insts:
            si = ins.sync_info
            if (
                si is not None
                and len(si.on_wait) > 1
                and str(ins.opcode) not in _MULTIWAIT_OK
            ):
                waits = list(si.on_wait)
                for w in waits[:-1]:
                    new.append(
                        mybir.InstNoOp(
                            name=nc.get_next_instruction_name(),
                            engine=ins.engine,
                            sync_info=mybir.SyncInfo(on_wait=[w], on_update=[]),
                            bass_nofuse=True,
                        )
                    )
                ins.sync_info = mybir.SyncInfo(
                    on_wait=[waits[-1]], on_update=list(si.on_update)
                )
                changed = True
            new.append(ins)
        if changed:
            insts[:] = new


def _build_program(phase=5):
    # phase: 1=qkv proj, 2=+V', 3=+scores/exp, 4=+AV/normalize, 5=full (debug aid)
    nc = bass.Bass()
    xT = nc.dram_tensor("xT", [D, L], mybir.dt.bfloat16, kind="ExternalInput")
    wqkvT = nc.dram_tensor("wqkvT", [D, PACK], mybir.dt.bfloat16, kind="ExternalInput")
    woutT = nc.dram_tensor("woutT", [128, 2, D], mybir.dt.bfloat16, kind="ExternalInput")
    wvT = nc.dram_tensor("wvT", [D, M2], mybir.dt.bfloat16, kind="ExternalInput")
    pout = nc.dram_tensor("pout", [L, D], mybir.dt.bfloat16, kind="ExternalOutput")

    with tile.TileContext(nc) as tc:
        with (
            tc.tile_pool(name="persist", bufs=1) as persist,
            tc.tile_pool(name="small", bufs=4) as small,
            tc.tile_pool(name="pp", bufs=2, space=bass.MemorySpace.PSUM) as pp,
            tc.tile_pool(name="pav", bufs=1, space=bass.MemorySpace.PSUM) as pav,
            tc.tile_pool(name="dscr", bufs=2, space="DRAM") as dscr,
        ):
            s_xT = persist.tile([128, DK, L], mybir.dt.bfloat16)
            s_w = persist.tile([128, DK, PACK], mybir.dt.bfloat16)
            xTr = xT.rearrange("(c p) l -> p c l", p=128)
            wTr = wqkvT.rearrange("(c p) m -> p c m", p=128)
            # x split into halves so the first proj units wait on half the
            # bytes; wv early (V' starts at c=0 of head 0's loop); wout last
            for dk in range(DK):
                nc.sync.dma_start(out=s_w[:, dk, :], in_=wTr[:, dk, :])
                nc.sync.dma_start(
                    out=s_xT[:, dk, 0:1024], in_=xTr[:, dk, 0:1024]
                )
            s_wv = persist.tile([128, DK, M2], mybir.dt.bfloat16)
            wvr = wvT.rearrange("(c p) m -> p c m", p=128)
            for d0 in range(0, DK, 2):
                nc.sync.dma_start(
                    out=s_wv[:, d0:d0 + 2, :], in_=wvr[:, d0:d0 + 2, :]
                )
            for dk in range(DK):
                # x upper halves land on fresh queues right behind the lower
                # halves; needed by the 2nd prologue proj unit (~15us)
                nc.sync.dma_start(
                    out=s_xT[:, dk, 1024:2048], in_=xTr[:, dk, 1024:2048]
                )
            del xTr
            # wout is consumed only by the final output projection: load last
            s_wout = persist.tile([128, 2, D], mybir.dt.bfloat16)
            nc.sync.dma_start(out=s_wout, in_=woutT[:])
            s_qkvT = persist.tile([128, MCH, L], mybir.dt.bfloat16)
            s_vp = persist.tile([128, NK, NH, HD + 1], mybir.dt.bfloat16)
            # per-key q-bias term SCALE*(bq_j . K_j[k]), fed to exp as a
            # per-partition activation bias (zeros when bqkv is zero)
            s_bqk = persist.tile([128, NK, NH], mybir.dt.float32)
            s_at = persist.tile([128, 2, L], mybir.dt.bfloat16)
            s_u65 = persist.tile([65, L], mybir.dt.float32)
            s_tmp64 = persist.tile([64, L], mybir.dt.bfloat16)
            s_identf = persist.tile([128, 128], mybir.dt.float32)
            make_identity(nc, s_identf)
            s_rqt = persist.tile([16, 128], mybir.dt.bfloat16)
            # recip row [1, L] (bf16); partition-broadcast via K=1 matmuls
            # (tail) or a stride-0 DMA broadcast into s_rbc (pipelined norms)
            s_rbc = persist.tile([64, L], mybir.dt.bfloat16)
            # unused tail rows of the A^T packing: zero so the K=128
            # output-projection matmul contracts them against zero W rows
            nc.vector.memset(s_at[64:128, 1, :], 0.0)

            # zero halves of the packed chunks (scores contraction padding).
            # k-bias shifts scores by a per-query constant -> cancels in
            # softmax; q-bias arrives via the exp activation bias (s_bqk);
            # v-bias is folded into bout on the host.
            nc.gpsimd.memset(s_qkvT[64:128, 0, :], 0.0)
            nc.gpsimd.memset(s_qkvT[0:64, 1, :], 0.0)
            nc.gpsimd.memset(s_qkvT[64:128, 2, :], 0.0)
            nc.gpsimd.memset(s_qkvT[64:128, 4, :], 0.0)

            # KQ projection: qkvT[m, l] = sum_d wqkvT[d, m] * xT[d, l]
            def emit_proj(u, nh):
                m, w_lo, mm, ob = PROJ_UNITS[u]
                acc = pp.tile([128, 1024], mybir.dt.float32, tag="big")
                for dk in range(DK):
                    for nn in range(2):
                        nc.tensor.matmul(
                            acc[ob:ob + mm, nn * 512:(nn + 1) * 512],
                            s_w[:, dk, w_lo:w_lo + mm],
                            s_xT[:, dk, nh * 1024 + nn * 512: nh * 1024 + (nn + 1) * 512],
                            start=(dk == 0),
                            stop=(dk == DK - 1),
                        )
                nc.vector.tensor_copy(
                    out=s_qkvT[ob:ob + mm, m, nh * 1024:(nh + 1) * 1024],
                    in_=acc[ob:ob + mm, :],
                )

            # only head 0's K/Q chunks up front (chunk 3 also carries Q1);
            # the rest interleave into head 0's c-loop so exp starts earlier.
            # nh=0 units first: the x upper halves are still in flight
            for nh in range(2):
                for u in (0, 3):
                    emit_proj(u, nh)
            proj_rest = [(1, 0), (1, 1), (4, 0), (4, 1),
                         (2, 0), (2, 1)]

            def k_pad(h):   # [128, L]: K_h^T packed against zeros
                return s_qkvT[:, KCH[h], :]

            def q_ext(h):   # [128, L]: Q_h^T (zero half kills the co-packed K)
                return s_qkvT[:, QCH[h], :]

            if phase < 5:
                ob0 = small.tile([128, D], mybir.dt.float32, tag="ob", bufs=3)
                nc.vector.memset(ob0, 0.0)
                nc.sync.dma_start(out=pout[0:128, :], in_=ob0)

            # ones column per head so A@V' also yields the softmax denominator
            if phase >= 2:
                nc.vector.memset(s_vp[:, :, :, HD:HD + 1], 1.0)

            def emit_vdirect(c):
                # V' built by a direct [l,d]-orientation projection: one
                # x^T-stationary matmul chain per key chunk (no transposes).
                # Columns 192:195 carry the per-key q-bias terms.
                # Uses the "av" PSUM slot — free during head 0's c-loop.
                vd = pav.tile([128, 2048], mybir.dt.float32, tag="av")
                for dk in range(DK):
                    nc.tensor.matmul(
                        vd[:, 0:M2],
                        s_xT[:, dk, c * 128:(c + 1) * 128],
                        s_wv[:, dk, :],
                        start=(dk == 0),
                        stop=(dk == DK - 1),
                    )
                nc.vector.tensor_copy(
                    out=s_vp[:, c, :, 0:HD],
                    in_=vd[:, 0:M].rearrange("p (j d) -> p j d", d=HD),
                )
                nc.vector.tensor_copy(
                    out=s_bqk[:, c, :], in_=vd[:, M:M2],
                )

            # Heads are software-pipelined: the c-loop of head j emits the
            # scores+exp for head j INTERLEAVED with the AV matmuls of head
            # j-1 (exp throttles scores via the sc slots; AV fills the PE
            # gaps).  E^T lives in a 17-chunk ring: exp(j,c) writes the slot
            # one behind the slot AV(j-1,c) reads.
            ER = NK + 1
            s_er = persist.tile([128, ER, L], mybir.dt.bfloat16)

            def eslot(j, c):
                return (NK * j + c) % ER

            def emit_scores(j, c):
                for qh in range(2):
                    sc = pp.tile([128, 1024], mybir.dt.float32, tag="big")
                    for nn in range(2):
                        nc.tensor.matmul(
                            sc[:, nn * 512:(nn + 1) * 512],
                            k_pad(j)[:, c * 128:(c + 1) * 128],
                            q_ext(j)[:, qh * 1024 + nn * 512: qh * 1024 + (nn + 1) * 512],
                            start=True,
                            stop=True,
                        )
                    nc.scalar.activation(
                        out=s_er[:, eslot(j, c), qh * 1024:(qh + 1) * 1024],
                        in_=sc,
                        func=mybir.ActivationFunctionType.Exp,
                        scale=SCALE,
                        bias=s_bqk[:, c, j:j + 1],
                    )

            def emit_av(j, c, av):
                # A'^T = V'^T.T @ E^T accumulated over key chunks:
                # rows 0:64 = unnormalized A^T, row 64 = softmax denominator.
                for nn in range(4):
                    nc.tensor.matmul(
                        av[0:HD + 1, nn * 512:(nn + 1) * 512],
                        s_vp[:, c, j, :],
                        s_er[:, eslot(j, c), nn * 512:(nn + 1) * 512],
                        start=(c == 0),
                        stop=(c == NK - 1),
                    )

            # -- normalize, staged so the recip chain latency hides under the
            # next head's c-loop (only PE-free stages run right after AV) --
            def emit_norm_pre(j, av):
                # evacuate U and den together -> releases the av PSUM slot so
                # the next head's AV matmuls can start during normalize
                nc.vector.tensor_copy(out=s_u65, in_=av[0:HD + 1, :])
                # den row -> q-partitioned [128,16] via a DRAM bounce (DVE
                # recip is ~6 cyc/elem; a [1,L] row would be single-lane)
                rq = small.tile([128, NQ], mybir.dt.float32, tag="rq")
                dden = dscr.tile([1, L], mybir.dt.float32, tag="dden")
                nc.sync.dma_start(out=dden, in_=s_u65[64:65, :])
                nc.sync.dma_start(
                    out=rq, in_=dden.rearrange("a (i p) -> (a p) i", p=128)
                )
                nc.vector.reciprocal(rq, rq)
                return rq

            def emit_norm_post1(j, rq):
                # recip back to a [1,L] row: [128,16] -T-> [16,128] -> DRAM
                # bounce (flat, so the final read is one descriptor)
                rqt_p = pp.tile([16, 128], mybir.dt.float32, tag="big")
                nc.tensor.transpose(rqt_p, rq, s_identf)
                nc.vector.tensor_copy(out=s_rqt, in_=rqt_p)
                drqt = dscr.tile([16, 128], mybir.dt.bfloat16, tag="drqt")
                nc.sync.dma_start(out=drqt, in_=s_rqt[:])
                # stride-0 partition broadcast on the (idle) DMA engines:
                # no PE matmuls, no psum slots taken from the score ring
                nc.sync.dma_start(
                    out=s_rbc,
                    in_=drqt.rearrange("i p -> () (i p)").broadcast_to([64, L]),
                )

            def emit_norm_post2(j):
                base = (j * HD) % 128
                ch = (j * HD) // 128
                for half in range(2):
                    sl = slice(half * 1024, (half + 1) * 1024)
                    if base == 0:
                        nc.vector.tensor_mul(
                            out=s_at[0:HD, ch, sl],
                            in0=s_u65[0:HD, sl],
                            in1=s_rbc[0:HD, sl],
                        )
                    else:
                        nc.vector.tensor_mul(
                            out=s_tmp64[:, sl],
                            in0=s_u65[0:HD, sl],
                            in1=s_rbc[0:HD, sl],
                        )
                if base != 0:
                    nc.sync.dma_start(
                        out=s_at[base:base + HD, ch, :], in_=s_tmp64[:, :]
                    )

            def emit_norm_tail(j, av):
                # last head: normalization is DEFERRED through the output
                # projection (1/den applied as a per-partition evac scale,
                # since queries sit on partitions there).  Only den -> rq
                # and the raw U evac remain; the kc0 output-projection
                # matmuls below overlap this whole chain.
                nc.scalar.copy(out=s_u65[64:65, 0:1024], in_=av[64:65, 0:1024])
                nc.vector.tensor_copy(
                    out=s_u65[64:65, 1024:2048], in_=av[64:65, 1024:2048]
                )
                # rqp in the av region (aliases av: writes wait its reads)
                rqp = pav.tile([128, 16], mybir.dt.float32, tag="av")
                for i in range(NQ):
                    # 1x1 "identity" = the diagonal element at partition 64,
                    # so fmap and weight share a base partition
                    nc.tensor.transpose(
                        rqp[:, i:i + 1],
                        s_u65[64:65, 128 * i:128 * (i + 1)],
                        s_identf[64:65, 64:65],
                    )
                rq = small.tile([128, NQ], mybir.dt.float32, tag="rq")
                nc.vector.reciprocal(rq[:, 0:8], rqp[:, 0:8])
                nc.vector.reciprocal(rq[:, 8:16], rqp[:, 8:16])
                # raw U2^T -> s_at chunk 1 (unnormalized; rows 64:128 zero)
                nc.vector.tensor_copy(out=s_at[0:64, 1, :], in_=av[0:64, :])
                return rq

            if phase >= 3:
                av = None
                pending = None   # (j, rq) of a norm awaiting its post stages
                for j in range(NH):
                    if j > 0 and phase >= 4:
                        av = pav.tile([128, L], mybir.dt.float32, tag="av")
                    for c in range(NK):
                        # vdirect(c) must precede scores(j=0,c): its evac
                        # writes the q-bias terms the exp bias reads
                        if j == 0 and phase >= 2:
                            emit_vdirect(c)
                        emit_scores(j, c)
                        if j == 0 and c % 2 == 0 and proj_rest:
                            emit_proj(*proj_rest.pop(0))
                        if j > 0 and phase >= 4:
                            emit_av(j - 1, c, av)
                        if pending is not None:
                            if c == 2:
                                emit_norm_post1(*pending)
                            elif c == 6:
                                emit_norm_post2(pending[0])
                                pending = None
                    if j > 0 and phase >= 4:
                        pending = (j - 1, emit_norm_pre(j - 1, av))
                if phase >= 4:
                    av = pav.tile([128, L], mybir.dt.float32, tag="av")
                    for c in range(NK):
                        emit_av(NH - 1, c, av)
                        if pending is not None:
                            if c == 2:
                                emit_norm_post1(*pending)
                            elif c == 6:
                                emit_norm_post2(pending[0])
                                pending = None
                    # pre-issue the first two kc0 groups (they depend only
                    # on heads 0/1, already normalized) so the PE rolls
                    # straight from AV into the output projection while the
                    # den->rq chain resolves
                    ots1 = {}

                    def emit_g1(qc):
                        ot1 = pp.tile([128, 1024], mybir.dt.float32, tag="big")
                        for n0, nlen in ((0, 512), (512, 256)):
                            nc.tensor.matmul(
                                ot1[:, n0:n0 + nlen],
                                s_at[:, 0, qc * 128:(qc + 1) * 128],
                                s_wout[:, 0, n0:n0 + nlen],
                                start=True,
                                stop=True,
                            )
                        return ot1

                    for qc in (0, 1):
                        ots1[qc] = emit_g1(qc)
                    rq_t = emit_norm_tail(NH - 1, av)

            # Row-parallel output projection: two psum groups per chunk —
            # G1 = normalized heads 0/1 (kc0), G2 = raw U2 (kc1).  The evac
            # applies 1/den to G2 as a per-partition ACT scale (queries on
            # partitions), then the DVE adds G1, so nothing waits on a
            # normalize of head 2's A.
            for qc in range(NQ if phase >= 5 else 0):
                ot1 = ots1.get(qc) or emit_g1(qc)
                ot2 = pav.tile([128, 1024], mybir.dt.float32, tag="av")
                for n0, nlen in ((0, 512), (512, 256)):
                    nc.tensor.matmul(
                        ot2[:, n0:n0 + nlen],
                        s_at[:, 1, qc * 128:(qc + 1) * 128],
                        s_wout[:, 1, n0:n0 + nlen],
                        start=True,
                        stop=True,
                    )
                ob = small.tile([128, D], mybir.dt.bfloat16, tag="ob", bufs=4)
                nc.scalar.activation(
                    out=ob,
                    in_=ot2[:, 0:D],
                    func=mybir.ActivationFunctionType.Copy,
                    scale=rq_t[:, qc:qc + 1],
                )
                nc.vector.tensor_add(out=ob, in0=ob, in1=ot1[:, 0:D])
                nc.sync.dma_start(
                    out=pout[qc * 128:(qc + 1) * 128, :], in_=ob
                )
    _split_multi_waits(nc)
    return nc


def _get_program():
    global _PROGRAM
    if _PROGRAM is None:
        _PROGRAM = _build_program()
    return _PROGRAM


def _make_in_maps(x, Wqkv, bqkv, Wout):
    in_maps = []
    for core in range(NCORES):
        b = core // GROUPS
        g = core % GROUPS
        heads = list(range(g * NH, (g + 1) * NH))
        # packed row r = 128*chunk + p; see layout comment at top.
        # Biases need no extra device matmuls: the k-bias shifts every score
        # for a query by a constant (softmax-invariant, dropped); the q-bias
        # term SCALE*(bq_j . K_j[k]) rides as 3 extra V'-proj columns fed to
        # exp as a per-partition bias; the v-bias is folded into bout.
        wpack = np.zeros((PACK, D), np.float32)   # [packed_row, d_in]
        wv = np.zeros((M2, D), np.float32)
        # K0 at rows 0:64, K1 at 192:256 (upper half of chunk 1), K2 at
        # 256:320; Q0|Q1 dense at 384:512; Q2 at 512:576
        k_lo = {0: 0, 1: 192, 2: 256}
        q_lo = {0: 384, 1: 448, 2: 512}
        for j, h in enumerate(heads):
            wk_h = Wqkv[D + h * HD: D + (h + 1) * HD]
            bq_h = bqkv[h * HD: (h + 1) * HD]
            wpack[k_lo[j]: k_lo[j] + HD] = wk_h
            wpack[q_lo[j]: q_lo[j] + HD] = Wqkv[h * HD: (h + 1) * HD]
            wv[j * HD: (j + 1) * HD] = Wqkv[2 * D + h * HD: 2 * D + (h + 1) * HD]
            wv[M + j] = SCALE * (bq_h @ wk_h)
        wqkvT_c = np.ascontiguousarray(wpack.T).astype(BF)
        wvT_c = np.ascontiguousarray(wv.T).astype(BF)
        xT_c = np.ascontiguousarray(x[b].T).astype(BF)
        wo = Wout[:, g * M:(g + 1) * M].T.astype(np.float32)  # [192, 768]
        woutT_c = np.zeros((128, 2, D), np.float32)
        woutT_c[:, 0, :] = wo[:128]
        woutT_c[:64, 1, :] = wo[128:]
        in_maps.append({
            "xT": xT_c,
            "wqkvT": wqkvT_c,
            "woutT": woutT_c.astype(BF),
            "wvT": wvT_c,
        })
    return in_maps


def _run(x, mask, Wqkv, bqkv, Wout, bout, trace=False):
    # mask is all-ones for this problem (spec fill: ones) -> softmax unmasked.
    x = np.asarray(x, np.float32)
    Wqkv = np.asarray(Wqkv, np.float32)
    bqkv = np.asarray(bqkv, np.float32)
    Wout = np.asarray(Wout, np.float32)
    bout = np.asarray(bout, np.float32)
    nc = _get_program()
    in_maps = _make_in_maps(x, Wqkv, bqkv, Wout)
    res = run_bass_kernel_spmd(nc, in_maps, list(range(NCORES)), trace=trace)
    out = np.zeros((B, L, D), np.float32)
    for core in range(NCORES):
        out[core // GROUPS] += np.asarray(res.results[core]["pout"], np.float32)
    # v-bias folded here: softmax rows sum to 1, so A @ (V + 1 bv^T) @ Wout^T
    # = A V Wout^T + bv @ Wout^T
    out += (bout + Wout @ bqkv[2 * D:3 * D])[None, None, :]
    return out, res


def kernel(x, mask, Wqkv, bqkv, Wout, bout):
    out, _ = _run(x, mask, Wqkv, bqkv, Wout, bout, trace=False)
    return out



# revision 57
# speedup vs baseline: 1.0626x; 1.0626x over previous
"""Multi-head self-attention (B=2, L=2048, D=768, H=12) on 8 TRN2 cores.

Sharding: data-parallel over batch (2 groups of 4 cores), tensor-parallel
over heads within each group (3 heads/core).  Each core computes the KQ
projection for its heads (packed into 5 half-dense 128-row chunks), V' in
[key, dim] orientation, full softmax attention, and a row-parallel partial
of the output projection.  The host sums the 4 partials per batch
(the row-parallel all-reduce) and adds the output bias.

qkv biases cost no device matmuls: the k-bias cancels in softmax, the
q-bias rides as extra V'-proj columns fed to exp as a per-partition
activation bias, and the v-bias folds into bout on the host (softmax rows
sum to 1).  All matmuls run in bf16 with fp32 PSUM accumulation; softmax
exp runs in fp32 on the scalar engine; the output partials are written in
bf16.  The softmax normalization hides its reciprocal-broadcast chain
under the next head's score loop.
Measured end-to-end L2 relative error vs the fp32 reference: ~5.7e-3.
"""

import sys

sys.path.insert(0, "/opt/trn_rl_repo")

import numpy as np
import ml_dtypes

import concourse.bass as bass
import concourse.mybir as mybir
import concourse.tile as tile
from concourse.bass_utils import run_bass_kernel_spmd
from concourse.masks import make_identity

B, L, D = 2, 2048, 768
H, HD = 12, 64
NCORES = 8
GROUPS = 4          # cores per batch
NH = H // GROUPS    # heads per core
M = NH * HD         # 192: packed width of one section (K/Q/V)
M2 = M + NH         # V'-proj width incl. the 3 q-bias columns
# Packed KQ layout (per core), 5 chunks of 128 rows:
#   chunk 0: [K0 | 0]  chunk 1: [0 | K1]  chunk 2: [K2 | 0]
#   chunk 3: [Q0 | Q1] (dense)            chunk 4: [Q2 | 0]
# Scores for head j contract over K=128 partitions (the PE runs K=64
# matmuls at half clock permanently): the zero half sits on whichever
# side (K or Q) makes the pairing contamination-free.
PACK = 5 * 128
DK = D // 128       # 6 contraction chunks
MCH = 5             # row-chunks of the packed KQ output
NQ = L // 128       # 16 query chunks
NK = L // 128       # 16 key chunks
SCALE = HD ** -0.5
BF = ml_dtypes.bfloat16
# proj emit units: (chunk, w_col_lo, n_rows, out_base_partition)
PROJ_UNITS = {
    0: (0, 0, 64, 0),      # K0
    1: (1, 192, 64, 64),   # K1 -> upper partitions
    2: (2, 256, 64, 0),    # K2
    3: (3, 384, 128, 0),   # Q0|Q1 dense
    4: (4, 512, 64, 0),    # Q2
}
KCH = {0: 0, 1: 1, 2: 2}   # scores lhsT chunk per head
QCH = {0: 3, 1: 3, 2: 4}   # scores moving chunk per head

_PROGRAM = None

# Opcodes whose walrus codegen accepts multiple sync waits (queue-level ops).
_MULTIWAIT_OK = {"EventSemaphore", "Call", "UnconditionalBranch",
                 "ConditionalBranch", "RegisterMove"}


def _split_multi_waits(nc):
    """This walrus build encodes at most ONE semaphore wait per TPB
    instruction (setupSyncWait: "Too many sync wait commands").  Tile's
    add_semaphores freely emits several.  Hoist all but one wait onto
    same-engine NoOps placed immediately before the instruction — engine
    streams execute in block order, so the stall semantics are identical.
    """
    import concourse.mybir as mybir  # local alias

    for bb in nc.main_func.blocks:
        insts = bb.instructions
        new = []
        changed = False
        for ins in insts:
            si = ins.sync_info
            if (
                si is not None
                and len(si.on_wait) > 1
                and str(ins.opcode) not in _MULTIWAIT_OK
            ):
                waits = list(si.on_wait)
                for w in waits[:-1]:
                    new.append(
                        mybir.InstNoOp(
                            name=nc.get_next_instruction_name(),
                            engine=ins.engine,
                            sync_info=mybir.SyncInfo(on_wait=[w], on_update=[]),
                            bass_nofuse=True,
                        )
                    )
                ins.sync_info = mybir.SyncInfo(
                    on_wait=[waits[-1]], on_update=list(si.on_update)
                )
                changed = True
            new.append(ins)
        if changed:
            insts[:] = new


def _build_program(phase=5):
    # phase: 1=qkv proj, 2=+V', 3=+scores/exp, 4=+AV/normalize, 5=full (debug aid)
    nc = bass.Bass()
    xT = nc.dram_tensor("xT", [D, L], mybir.dt.bfloat16, kind="ExternalInput")
    wqkvT = nc.dram_tensor("wqkvT", [D, PACK], mybir.dt.bfloat16, kind="ExternalInput")
    woutT = nc.dram_tensor("woutT", [128, 2, D], mybir.dt.bfloat16, kind="ExternalInput")
    wvT = nc.dram_tensor("wvT", [D, M2], mybir.dt.bfloat16, kind="ExternalInput")
    pout = nc.dram_tensor("pout", [L, D], mybir.dt.bfloat16, kind="ExternalOutput")

    with tile.TileContext(nc) as tc:
        with (
            tc.tile_pool(name="persist", bufs=1) as persist,
            tc.tile_pool(name="small", bufs=4) as small,
            tc.tile_pool(name="pp", bufs=2, space=bass.MemorySpace.PSUM) as pp,
            tc.tile_pool(name="pav", bufs=1, space=bass.MemorySpace.PSUM) as pav,
            tc.tile_pool(name="dscr", bufs=2, space="DRAM") as dscr,
        ):
            s_xT = persist.tile([128, DK, L], mybir.dt.bfloat16)
            s_w = persist.tile([128, DK, PACK], mybir.dt.bfloat16)
            xTr = xT.rearrange("(c p) l -> p c l", p=128)
            wTr = wqkvT.rearrange("(c p) m -> p c m", p=128)
            # x lower halves split into 512-col pieces across more queues:
            # the proj chain's (dk, nn) matmuls each wait on just one piece;
            # wv early (V' starts at c=0 of head 0's loop); wout last
            for dk in range(DK):
                nc.sync.dma_start(out=s_w[:, dk, :], in_=wTr[:, dk, :])
                nc.sync.dma_start(
                    out=s_xT[:, dk, 0:512], in_=xTr[:, dk, 0:512]
                )
                nc.sync.dma_start(
                    out=s_xT[:, dk, 512:1024], in_=xTr[:, dk, 512:1024]
                )
            s_wv = persist.tile([128, DK, M2], mybir.dt.bfloat16)
            wvr = wvT.rearrange("(c p) m -> p c m", p=128)
            for d0 in range(0, DK, 2):
                nc.sync.dma_start(
                    out=s_wv[:, d0:d0 + 2, :], in_=wvr[:, d0:d0 + 2, :]
                )
            for dk in range(DK):
                # x upper halves land on fresh queues right behind the lower
                # halves; needed by the 2nd prologue proj unit (~15us)
                nc.sync.dma_start(
                    out=s_xT[:, dk, 1024:2048], in_=xTr[:, dk, 1024:2048]
                )
            del xTr
            # wout is consumed only by the final output projection: load last
            s_wout = persist.tile([128, 2, D], mybir.dt.bfloat16)
            nc.sync.dma_start(out=s_wout, in_=woutT[:])
            s_qkvT = persist.tile([128, MCH, L], mybir.dt.bfloat16)
            s_vp = persist.tile([128, NK, NH, HD + 1], mybir.dt.bfloat16)
            # per-key q-bias term SCALE*(bq_j . K_j[k]), fed to exp as a
            # per-partition activation bias (zeros when bqkv is zero)
            s_bqk = persist.tile([128, NK, NH], mybir.dt.float32)
            s_at = persist.tile([128, 2, L], mybir.dt.bfloat16)
            s_u65 = persist.tile([65, L], mybir.dt.float32)
            s_tmp64 = persist.tile([64, L], mybir.dt.bfloat16)
            s_identf = persist.tile([128, 128], mybir.dt.float32)
            make_identity(nc, s_identf)
            s_rqt = persist.tile([16, 128], mybir.dt.bfloat16)
            # recip row [1, L] (bf16); partition-broadcast via K=1 matmuls
            # (tail) or a stride-0 DMA broadcast into s_rbc (pipelined norms)
            s_rrow = persist.tile([1, L], mybir.dt.bfloat16)
            s_rbc = persist.tile([64, L], mybir.dt.bfloat16)
            s_ones64 = persist.tile([1, 64], mybir.dt.bfloat16)
            nc.gpsimd.memset(s_ones64, 1.0)
            # unused tail rows of the A^T packing: zero so the K=128
            # output-projection matmul contracts them against zero W rows
            nc.vector.memset(s_at[64:128, 1, :], 0.0)

            # zero halves of the packed chunks (scores contraction padding).
            # k-bias shifts scores by a per-query constant -> cancels in
            # softmax; q-bias arrives via the exp activation bias (s_bqk);
            # v-bias is folded into bout on the host.
            nc.gpsimd.memset(s_qkvT[64:128, 0, :], 0.0)
            nc.gpsimd.memset(s_qkvT[0:64, 1, :], 0.0)
            nc.gpsimd.memset(s_qkvT[64:128, 2, :], 0.0)
            nc.gpsimd.memset(s_qkvT[64:128, 4, :], 0.0)

            # KQ projection: qkvT[m, l] = sum_d wqkvT[d, m] * xT[d, l]
            def emit_proj(u, nh):
                m, w_lo, mm, ob = PROJ_UNITS[u]
                acc = pp.tile([128, 1024], mybir.dt.float32, tag="big")
                for dk in range(DK):
                    for nn in range(2):
                        nc.tensor.matmul(
                            acc[ob:ob + mm, nn * 512:(nn + 1) * 512],
                            s_w[:, dk, w_lo:w_lo + mm],
                            s_xT[:, dk, nh * 1024 + nn * 512: nh * 1024 + (nn + 1) * 512],
                            start=(dk == 0),
                            stop=(dk == DK - 1),
                        )
                nc.vector.tensor_copy(
                    out=s_qkvT[ob:ob + mm, m, nh * 1024:(nh + 1) * 1024],
                    in_=acc[ob:ob + mm, :],
                )

            # only head 0's K/Q chunks up front (chunk 3 also carries Q1);
            # the rest interleave into head 0's c-loop so exp starts earlier.
            # nh=0 units first: the x upper halves are still in flight
            for nh in range(2):
                for u in (0, 3):
                    emit_proj(u, nh)
            proj_rest = [(1, 0), (1, 1), (4, 0), (4, 1),
                         (2, 0), (2, 1)]

            def k_pad(h):   # [128, L]: K_h^T packed against zeros
                return s_qkvT[:, KCH[h], :]

            def q_ext(h):   # [128, L]: Q_h^T (zero half kills the co-packed K)
                return s_qkvT[:, QCH[h], :]

            if phase < 5:
                ob0 = small.tile([128, D], mybir.dt.float32, tag="ob", bufs=3)
                nc.vector.memset(ob0, 0.0)
                nc.sync.dma_start(out=pout[0:128, :], in_=ob0)

            # ones column per head so A@V' also yields the softmax denominator
            if phase >= 2:
                nc.vector.memset(s_vp[:, :, :, HD:HD + 1], 1.0)

            def emit_vdirect(c):
                # V' built by a direct [l,d]-orientation projection: one
                # x^T-stationary matmul chain per key chunk (no transposes).
                # Columns 192:195 carry the per-key q-bias terms.
                # Uses the "av" PSUM slot — free during head 0's c-loop.
                vd = pav.tile([128, 2048], mybir.dt.float32, tag="av")
                for dk in range(DK):
                    nc.tensor.matmul(
                        vd[:, 0:M2],
                        s_xT[:, dk, c * 128:(c + 1) * 128],
                        s_wv[:, dk, :],
                        start=(dk == 0),
                        stop=(dk == DK - 1),
                    )
                nc.vector.tensor_copy(
                    out=s_vp[:, c, :, 0:HD],
                    in_=vd[:, 0:M].rearrange("p (j d) -> p j d", d=HD),
                )
                nc.vector.tensor_copy(
                    out=s_bqk[:, c, :], in_=vd[:, M:M2],
                )

            # Heads are software-pipelined: the c-loop of head j emits the
            # scores+exp for head j INTERLEAVED with the AV matmuls of head
            # j-1 (exp throttles scores via the sc slots; AV fills the PE
            # gaps).  E^T lives in a 17-chunk ring: exp(j,c) writes the slot
            # one behind the slot AV(j-1,c) reads.
            ER = NK + 1
            s_er = persist.tile([128, ER, L], mybir.dt.bfloat16)

            def eslot(j, c):
                return (NK * j + c) % ER

            def emit_scores(j, c):
                for qh in range(2):
                    sc = pp.tile([128, 1024], mybir.dt.float32, tag="big")
                    for nn in range(2):
                        nc.tensor.matmul(
                            sc[:, nn * 512:(nn + 1) * 512],
                            k_pad(j)[:, c * 128:(c + 1) * 128],
                            q_ext(j)[:, qh * 1024 + nn * 512: qh * 1024 + (nn + 1) * 512],
                            start=True,
                            stop=True,
                        )
                    nc.scalar.activation(
                        out=s_er[:, eslot(j, c), qh * 1024:(qh + 1) * 1024],
                        in_=sc,
                        func=mybir.ActivationFunctionType.Exp,
                        scale=SCALE,
                        bias=s_bqk[:, c, j:j + 1],
                    )

            def emit_av(j, c, av):
                # A'^T = V'^T.T @ E^T accumulated over key chunks:
                # rows 0:64 = unnormalized A^T, row 64 = softmax denominator.
                for nn in range(4):
                    nc.tensor.matmul(
                        av[0:HD + 1, nn * 512:(nn + 1) * 512],
                        s_vp[:, c, j, :],
                        s_er[:, eslot(j, c), nn * 512:(nn + 1) * 512],
                        start=(c == 0),
                        stop=(c == NK - 1),
                    )

            # -- normalize, staged so the recip chain latency hides under the
            # next head's c-loop (only PE-free stages run right after AV) --
            def emit_norm_pre(j, av):
                # evacuate U and den together -> releases the av PSUM slot so
                # the next head's AV matmuls can start during normalize
                nc.vector.tensor_copy(out=s_u65, in_=av[0:HD + 1, :])
                # den row -> q-partitioned [128,16] via a DRAM bounce (DVE
                # recip is ~6 cyc/elem; a [1,L] row would be single-lane)
                rq = small.tile([128, NQ], mybir.dt.float32, tag="rq")
                dden = dscr.tile([1, L], mybir.dt.float32, tag="dden")
                nc.sync.dma_start(out=dden, in_=s_u65[64:65, :])
                nc.sync.dma_start(
                    out=rq, in_=dden.rearrange("a (i p) -> (a p) i", p=128)
                )
                nc.vector.reciprocal(rq, rq)
                return rq

            def emit_norm_post1(j, rq):
                # recip back to a [1,L] row: [128,16] -T-> [16,128] -> DRAM
                # bounce (flat, so the final read is one descriptor)
                rqt_p = pp.tile([16, 128], mybir.dt.float32, tag="big")
                nc.tensor.transpose(rqt_p, rq, s_identf)
                nc.vector.tensor_copy(out=s_rqt, in_=rqt_p)
                drqt = dscr.tile([16, 128], mybir.dt.bfloat16, tag="drqt")
                nc.sync.dma_start(out=drqt, in_=s_rqt[:])
                # stride-0 partition broadcast on the (idle) DMA engines:
                # no PE matmuls, no psum slots taken from the score ring
                nc.sync.dma_start(
                    out=s_rbc,
                    in_=drqt.rearrange("i p -> () (i p)").broadcast_to([64, L]),
                )

            def emit_norm_post2(j):
                base = (j * HD) % 128
                ch = (j * HD) // 128
                for half in range(2):
                    sl = slice(half * 1024, (half + 1) * 1024)
                    if j < NH - 1:
                        rbv = s_rbc[0:HD, sl]
                    else:
                        # tail: broadcast s_rrow with K=1 ones-matmuls (the
                        # PE is idle here and matmul latency beats a DMA)
                        rb = pp.tile([128, 1024], mybir.dt.float32, tag="big")
                        for nn in range(2):
                            nc.tensor.matmul(
                                rb[0:HD, nn * 512:(nn + 1) * 512],
                                s_ones64[0:1, :],
                                s_rrow[0:1, half * 1024 + nn * 512:
                                       half * 1024 + (nn + 1) * 512],
                                start=True,
                                stop=True,
                            )
                        rbv = rb[0:HD, :]
                    if base == 0:
                        nc.vector.tensor_mul(
                            out=s_at[0:HD, ch, sl],
                            in0=s_u65[0:HD, sl],
                            in1=rbv,
                        )
                    else:
                        nc.vector.tensor_mul(
                            out=s_tmp64[:, sl],
                            in0=s_u65[0:HD, sl],
                            in1=rbv,
                        )
                if base != 0:
                    nc.sync.dma_start(
                        out=s_at[base:base + HD, ch, :], in_=s_tmp64[:, :]
                    )

            def emit_norm_tail(j, av):
                # last head: nothing overlaps, so minimize chain latency:
                # on-chip transposes instead of DMA bounces, serial stages
                # split across both copy engines.  The recip chain (den row
                # -> transposes -> recip -> row) is emitted on DVE BEFORE the
                # big U evac so it isn't queued behind it.
                nc.scalar.copy(out=s_u65[64:65, 0:1024], in_=av[64:65, 0:1024])
                nc.vector.tensor_copy(
                    out=s_u65[64:65, 1024:2048], in_=av[64:65, 1024:2048]
                )
                rqp = pp.tile([128, 16], mybir.dt.float32, tag="big")
                for i in range(NQ):
                    # 1x1 "identity" = the diagonal element at partition 64,
                    # so fmap and weight share a base partition
                    nc.tensor.transpose(
                        rqp[:, i:i + 1],
                        s_u65[64:65, 128 * i:128 * (i + 1)],
                        s_identf[64:65, 64:65],
                    )
                rq = small.tile([128, NQ], mybir.dt.float32, tag="rq")
                nc.vector.reciprocal(rq[:, 0:8], rqp[:, 0:8])
                nc.vector.reciprocal(rq[:, 8:16], rqp[:, 8:16])
                # U evac on ACT: keeps the DVE free for the recip chain
                nc.scalar.copy(out=s_u65[0:64, :], in_=av[0:64, :])
                # recip columns -> [1,L] psum row halves in the free sc
                # slots (16 transposes, no DMA)
                rbrow0 = pp.tile([1, 1024], mybir.dt.float32, tag="big")
                rbrow1 = pp.tile([1, 1024], mybir.dt.float32, tag="big")
                rbrows = [rbrow0, rbrow1]
                for i in range(NQ):
                    nc.tensor.transpose(
                        rbrows[i // 8][0:1, 128 * (i % 8):128 * (i % 8 + 1)],
                        rq[:, i:i + 1],
                        s_identf,
                    )
                # psum row -> SBUF bf16, one half per copy engine
                nc.vector.tensor_copy(out=s_rrow[0:1, 0:1024], in_=rbrows[0][0:1, :])
                nc.scalar.copy(
                    out=s_rrow[0:1, 1024:2048], in_=rbrows[1][0:1, :]
                )
                emit_norm_post2(j)

            if phase >= 3:
                av = None
                pending = None   # (j, rq) of a norm awaiting its post stages
                for j in range(NH):
                    if j > 0 and phase >= 4:
                        av = pav.tile([128, L], mybir.dt.float32, tag="av")
                    for c in range(NK):
                        # vdirect(c) must precede scores(j=0,c): its evac
                        # writes the q-bias terms the exp bias reads
                        if j == 0 and phase >= 2:
                            emit_vdirect(c)
                        emit_scores(j, c)
                        if j == 0 and c % 2 == 0 and proj_rest:
                            emit_proj(*proj_rest.pop(0))
                        if j > 0 and phase >= 4:
                            emit_av(j - 1, c, av)
                        if pending is not None:
                            if c == 2:
                                emit_norm_post1(*pending)
                            elif c == 6:
                                emit_norm_post2(pending[0])
                                pending = None
                    if j > 0 and phase >= 4:
                        pending = (j - 1, emit_norm_pre(j - 1, av))
                if phase >= 4:
                    av = pav.tile([128, L], mybir.dt.float32, tag="av")
                    for c in range(NK):
                        emit_av(NH - 1, c, av)
                        if pending is not None:
                            if c == 2:
                                emit_norm_post1(*pending)
                            elif c == 6:
                                emit_norm_post2(pending[0])
                                pending = None
                    emit_norm_tail(NH - 1, av)

            # Row-parallel output projection partial: pout = A @ woutT.
            # Every 3rd chunk draws its psum tile from the (now free) av
            # region, giving 3 slots so evac latency never gates the matmuls
            for qc in range(NQ if phase >= 5 else 0):
                if qc % 3 == 2:
                    ot = pav.tile([128, 1024], mybir.dt.float32, tag="av")
                else:
                    ot = pp.tile([128, 1024], mybir.dt.float32, tag="big")
                for kc in range(2):
                    for n0, nlen in ((0, 512), (512, 256)):
                        nc.tensor.matmul(
                            ot[:, n0:n0 + nlen],
                            s_at[:, kc, qc * 128:(qc + 1) * 128],
                            s_wout[:, kc, n0:n0 + nlen],
                            start=(kc == 0),
                            stop=(kc == 1),
                        )
                ob = small.tile([128, D], mybir.dt.bfloat16, tag="ob", bufs=4)
                # split each evac across both copy engines so psum slot
                # turnaround beats the matmul rate
                nc.vector.tensor_copy(ob[:, 0:384], ot[:, 0:384])
                nc.scalar.copy(ob[:, 384:D], ot[:, 384:D])
                nc.sync.dma_start(
                    out=pout[qc * 128:(qc + 1) * 128, :], in_=ob
                )
    _split_multi_waits(nc)
    return nc


def _get_program():
    global _PROGRAM
    if _PROGRAM is None:
        _PROGRAM = _build_program()
    return _PROGRAM


def _make_in_maps(x, Wqkv, bqkv, Wout):
    in_maps = []
    for core in range(NCORES):
        b = core // GROUPS
        g = core % GROUPS
        heads = list(range(g * NH, (g + 1) * NH))
        # packed row r = 128*chunk + p; see layout comment at top.
        # Biases need no extra device matmuls: the k-bias shifts every score
        # for a query by a constant (softmax-invariant, dropped); the q-bias
        # term SCALE*(bq_j . K_j[k]) rides as 3 extra V'-proj columns fed to
        # exp as a per-partition bias; the v-bias is folded into bout.
        wpack = np.zeros((PACK, D), np.float32)   # [packed_row, d_in]
        wv = np.zeros((M2, D), np.float32)
        # K0 at rows 0:64, K1 at 192:256 (upper half of chunk 1), K2 at
        # 256:320; Q0|Q1 dense at 384:512; Q2 at 512:576
        k_lo = {0: 0, 1: 192, 2: 256}
        q_lo = {0: 384, 1: 448, 2: 512}
        for j, h in enumerate(heads):
            wk_h = Wqkv[D + h * HD: D + (h + 1) * HD]
            bq_h = bqkv[h * HD: (h + 1) * HD]
            wpack[k_lo[j]: k_lo[j] + HD] = wk_h
            wpack[q_lo[j]: q_lo[j] + HD] = Wqkv[h * HD: (h + 1) * HD]
            wv[j * HD: (j + 1) * HD] = Wqkv[2 * D + h * HD: 2 * D + (h + 1) * HD]
            wv[M + j] = SCALE * (bq_h @ wk_h)
        wqkvT_c = np.ascontiguousarray(wpack.T).astype(BF)
        wvT_c = np.ascontiguousarray(wv.T).astype(BF)
        xT_c = np.ascontiguousarray(x[b].T).astype(BF)
        wo = Wout[:, g * M:(g + 1) * M].T.astype(np.float32)  # [192, 768]
        woutT_c = np.zeros((128, 2, D), np.float32)
        woutT_c[:, 0, :] = wo[:128]
        woutT_c[:64, 1, :] = wo[128:]
        in_maps.append({
            "xT": xT_c,
            "wqkvT": wqkvT_c,
            "woutT": woutT_c.astype(BF),
            "wvT": wvT_c,
        })
    return in_maps


def _run(x, mask, Wqkv, bqkv, Wout, bout, trace=False):
    # mask is all-ones for this problem (spec fill: ones) -> softmax unmasked.
    x = np.asarray(x, np.float32)
    Wqkv = np.asarray(Wqkv, np.float32)
    bqkv = np.asarray(bqkv, np.float32)
    Wout = np.asarray(Wout, np.float32)
    bout = np.asarray(bout, np.float32)
    nc = _get_program()
    in_maps = _make_in_maps(x, Wqkv, bqkv, Wout)
    res = run_bass_kernel_spmd(nc, in_maps, list(range(NCORES)), trace=trace)
    out = np.zeros((B, L, D), np.float32)
    for core in range(NCORES):
        out[core // GROUPS] += np.asarray(res.results[core]["pout"], np.float32)
    # v-bias folded here: softmax rows sum to 1, so A @ (V + 1 bv^T) @ Wout^T
    # = A V Wout^T + bv @ Wout^T
    out += (bout + Wout @ bqkv[2 * D:3 * D])[None, None, :]
    return out, res


def kernel(x, mask, Wqkv, bqkv, Wout, bout):
    out, _ = _run(x, mask, Wqkv, bqkv, Wout, bout, trace=False)
    return out

